# revision 24
# baseline (speedup 1.0000x reference)
"""Trainium2 Bass kernel for BlockAxialDown (maxpool + axial attention + 1x1 conv + batchnorm).

Contract: kernel(**inputs) takes FULL unsharded inputs, returns FULL output.
Sharding: data-parallel over batch B=8 across 8 NeuronCores (1 image/core);
BatchNorm batch stats combined with a tiny (128,4) AllReduce; weights replicated.

Transposed-attention dataflow: dots are computed as dotsT[j,i] = k^T q so the
attention weights come out already in the (key, query) layout the AV matmul
needs -- no transposes anywhere.  Softmax denominators are produced by a
ones-matmul whose output is naturally broadcast across all 128 partitions;
normalization is one reciprocal + one multiply fused into the PSUM drain.
Conv runs once with y kept on-chip (bf16), BN affine applied after the
stats AllReduce.  Output returned as bf16, cast to f32 on host.
"""

import sys

import numpy as np

for _p in ("/opt/trn_rl_repo", "/root/.axon_site/_ro/trn_rl_repo"):
    if _p not in sys.path:
        sys.path.append(_p)

B, C, H, W = 8, 128, 256, 256
H2, W2 = 128, 128
E = 2 * C
NPOS = H2 * W2
NCORES = 8
BN_EPS = 1e-5
DH = C // 2
SCALE = DH ** -0.5

_CACHE = {}


def _build_program():
    import concourse.tile as tile
    from concourse import bacc, mybir
    from concourse.alu_op_type import AluOpType
    from contextlib import ExitStack

    F32 = mybir.dt.float32
    BF16 = mybir.dt.bfloat16
    AF = mybir.ActivationFunctionType
    P = 128

    nc = bacc.Bacc("TRN2", target_bir_lowering=False, debug=False, num_devices=NCORES)

    # ---- DRAM I/O ----
    x_d = nc.dram_tensor("x", [C, H, W], BF16, kind="ExternalInput").ap()
    wq_w_d = nc.dram_tensor("wq_w", [C, C], BF16, kind="ExternalInput").ap()
    wk_w_d = nc.dram_tensor("wk_w", [C, C], BF16, kind="ExternalInput").ap()
    wv_w_d = nc.dram_tensor("wv_w", [C, C], BF16, kind="ExternalInput").ap()
    wo_w_d = nc.dram_tensor("wo_w", [C, C], BF16, kind="ExternalInput").ap()
    wq_h_d = nc.dram_tensor("wq_h", [C, C], BF16, kind="ExternalInput").ap()
    wk_h_d = nc.dram_tensor("wk_h", [C, C], BF16, kind="ExternalInput").ap()
    wv_h_d = nc.dram_tensor("wv_h", [C, C], BF16, kind="ExternalInput").ap()
    wo_h_d = nc.dram_tensor("wo_h", [C, C], BF16, kind="ExternalInput").ap()
    bsum_d = nc.dram_tensor("bsum", [C, 1], F32, kind="ExternalInput").ap()
    convA_d = nc.dram_tensor("convA", [C, E], BF16, kind="ExternalInput").ap()
    convX_d = nc.dram_tensor("convX", [C, E], BF16, kind="ExternalInput").ap()
    gamma2_d = nc.dram_tensor("gamma2", [C, 2], F32, kind="ExternalInput").ap()
    beta2_d = nc.dram_tensor("beta2", [C, 2], F32, kind="ExternalInput").ap()
    out_d = nc.dram_tensor("out", [E, H2, W2], BF16, kind="ExternalOutput").ap()
    stats_in_d = nc.dram_tensor("stats_in", [P, 4], F32).ap()
    stats_out_d = nc.dram_tensor("stats_out", [P, 4], F32, addr_space="Shared").ap()

    with tile.TileContext(nc) as tc, ExitStack() as ctx:
        const = ctx.enter_context(tc.tile_pool(name="const", bufs=1))
        cube = ctx.enter_context(tc.tile_pool(name="cube", bufs=1))
        stage = ctx.enter_context(tc.tile_pool(name="stage", bufs=3))
        work = ctx.enter_context(tc.tile_pool(name="work", bufs=3))
        stats = ctx.enter_context(tc.tile_pool(name="stats", bufs=1))
        psum = ctx.enter_context(tc.tile_pool(name="psum", bufs=1, space="PSUM"))

        # ---- constants ----
        def cload(name, ap_d, shape, dt):
            t = const.tile(shape, dt, name=name)
            nc.sync.dma_start(out=t[:], in_=ap_d)
            return t

        wq_w = cload("wq_w_t", wq_w_d, [C, C], BF16)
        wk_w = cload("wk_w_t", wk_w_d, [C, C], BF16)
        wv_w = cload("wv_w_t", wv_w_d, [C, C], BF16)
        wo_w = cload("wo_w_t", wo_w_d, [C, C], BF16)
        wq_h = cload("wq_h_t", wq_h_d, [C, C], BF16)
        wk_h = cload("wk_h_t", wk_h_d, [C, C], BF16)
        wv_h = cload("wv_h_t", wv_h_d, [C, C], BF16)
        wo_h = cload("wo_h_t", wo_h_d, [C, C], BF16)
        bsum = cload("bsum_t", bsum_d, [C, 1], F32)
        convA = cload("convA_t", convA_d, [C, E], BF16)
        convX = cload("convX_t", convX_d, [C, E], BF16)
        gamma2 = cload("gamma2_t", gamma2_d, [C, 2], F32)
        beta2 = cload("beta2_t", beta2_d, [C, 2], F32)
        ones64 = const.tile([P, 64], BF16, name="ones64")
        nc.vector.memset(ones64[:], 1.0)

        xp = cube.tile([P, H2, W2], BF16)   # pooled input, channels on partitions
        acc = cube.tile([P, H2, W2], BF16)  # attention output accumulator
        xp_f = xp[:].rearrange("c h w -> c (h w)")
        acc_f = acc[:].rearrange("c h w -> c (h w)")

        # ---- phase 1: load + 2x2 maxpool ----
        xv = x_d.rearrange("c (n r) w -> c n r w", r=16)
        for i in range(H // 16):
            xin = stage.tile([P, 16, W], BF16, tag="xin")
            nc.sync.dma_start(out=xin[:], in_=xv[:, i])
            t = stage.tile([P, 16, W2], BF16, tag="wmax")
            xin4 = xin[:].rearrange("c r (w two) -> c r w two", two=2)
            nc.vector.tensor_max(t[:], xin4[:, :, :, 0], xin4[:, :, :, 1])
            t4 = t[:].rearrange("c (r2 two) w -> c r2 two w", two=2)
            nc.vector.tensor_max(xp[:, 8 * i:8 * i + 8, :], t4[:, :, 0, :], t4[:, :, 1, :])

        # ---- axial attention, 2-stage software pipeline over 64 groups ----
        # Front half of group g (projections, dots, exp) is emitted together
        # with the back half of group g-1 (denominators, AV, out-proj, acc) so
        # the in-order ACT/DVE queues never block on the current group's tail.
        NG = H2 // 4  # 32 groups per direction

        def grp(g):
            """(rhs_g, weights..., is_w_direction) for global group index g."""
            if g < NG:
                return (xp[:, 4 * g:4 * g + 4, :], wq_w, wk_w, wv_w, wo_w, True, g)
            gg = g - NG
            rhs = xp[:, :, 4 * gg:4 * gg + 4].rearrange("c h w -> c w h")
            return (rhs, wq_h, wk_h, wv_h, wo_h, False, gg)

        def front(g):
            rhs_g, wq, wk, wv, wo, _, _ = grp(g)
            qg_ps = psum.tile([P, 512], F32, tag="proj", bufs=3, name="qg_ps")
            nc.tensor.matmul(qg_ps[:], lhsT=wq[:], rhs=rhs_g, start=True, stop=True)
            kg_ps = psum.tile([P, 512], F32, tag="proj", bufs=3, name="kg_ps")
            nc.tensor.matmul(kg_ps[:], lhsT=wk[:], rhs=rhs_g, start=True, stop=True)
            qg = work.tile([P, 512], BF16, tag="qg")
            nc.scalar.copy(qg[:], qg_ps[:])
            kg = work.tile([P, 512], BF16, tag="kg")
            nc.vector.tensor_copy(kg[:], kg_ps[:])
            v_ps = psum.tile([P, 512], F32, tag="v", name="v_ps")
            for s in range(4):
                nc.tensor.matmul(v_ps[:, 128 * s:128 * s + 128], lhsT=rhs_g[:, s, :],
                                 rhs=wv[:], start=True, stop=True)
            vs = work.tile([P, 512], BF16, tag="vs", bufs=4)
            nc.scalar.copy(vs[:], v_ps[:])
            # dotsT[j, i] per (slice, head), head-major columns: col = 512*h + 128*s
            dT = psum.tile([P, 1024], F32, tag="dots", name="dT")
            for s in range(4):
                cs = slice(128 * s, 128 * s + 128)
                for h in range(2):
                    hp = slice(64 * h, 64 * h + 64)
                    nc.tensor.matmul(dT[:, 512 * h + 128 * s:512 * h + 128 * s + 128],
                                     lhsT=kg[hp, cs], rhs=qg[hp, cs],
                                     start=True, stop=True)
            e = work.tile([P, 1024], BF16, tag="e", bufs=4)
            nc.scalar.activation(e[:], dT[:], AF.Exp, scale=SCALE)
            return e, vs

        def back(g, e, vs):
            rhs_g, wq, wk, wv, wo, is_w, gg = grp(g)
            # softmax denominators, broadcast across partitions by the matmul
            bc = psum.tile([P, 512], F32, tag="bc", name="bc")
            nc.tensor.matmul(bc[0:64, :], lhsT=ones64[:], rhs=e[:, 0:512],
                             start=True, stop=True)
            nc.tensor.matmul(bc[64:128, :], lhsT=ones64[:], rhs=e[:, 512:1024],
                             start=True, stop=True, tile_position=(0, 64))
            rcp = work.tile([P, 512], F32, tag="rcp")
            nc.vector.reciprocal_approx_fast(rcp[:], bc[:])
            # attn @ v (unnormalized), then normalize while draining PSUM
            oT = psum.tile([P, 512], F32, tag="oT", name="oT")
            for s in range(4):
                nc.tensor.matmul(
                    oT[0:64, 128 * s:128 * s + 128],
                    lhsT=vs[:, 128 * s:128 * s + 64],
                    rhs=e[:, 128 * s:128 * s + 128],
                    start=True, stop=True)
                nc.tensor.matmul(
                    oT[64:128, 128 * s:128 * s + 128],
                    lhsT=vs[:, 128 * s + 64:128 * s + 128],
                    rhs=e[:, 512 + 128 * s:512 + 128 * s + 128],
                    start=True, stop=True, tile_position=(0, 64))
            og = work.tile([P, 512], BF16, tag="og")
            nc.vector.tensor_mul(og[:], oT[:], rcp[:])
            yg = psum.tile([P, 512], F32, tag="proj", bufs=3, name="yg_ps")
            nc.tensor.matmul(yg[:], lhsT=wo[:], rhs=og[:], start=True, stop=True)
            if is_w:
                # acc = yg_w + (bout_h + bout_w), contiguous write
                nc.scalar.activation(acc_f[:, 512 * gg:512 * (gg + 1)], yg[:],
                                     AF.Identity, bias=bsum[:, 0:1], scale=1.0)
            else:
                # accumulate transposed: acc[:, h, w] += yg[:, (s=w, i=h)]
                acc_sl = acc[:, :, 4 * gg:4 * gg + 4]
                yg_r = yg[:].rearrange("c (s i) -> c i s", s=4)
                nc.vector.tensor_add(acc_sl, acc_sl, yg_r)

        LAG = 3
        pend = []
        for g in range(2 * NG):
            pend.append((g, front(g)))
            if len(pend) > LAG:
                pg, pe = pend.pop(0)
                back(pg, *pe)
        for pg, pe in pend:
            back(pg, *pe)

        # ---- phase 3.5: relu over acc ----
        for j in range(4):
            sl = acc_f[:, 4096 * j:4096 * (j + 1)]
            nc.vector.tensor_scalar_max(sl, sl, 0.0)

        # ---- phase 4: conv (once), relu evac to y (over acc/xp), stats ----
        # y half 0 overwrites acc chunk-by-chunk, half 1 overwrites xp.
        y_f = [acc_f, xp_f]
        bnb = [stats.tile([P, 32, 6], F32, name=f"bnb{i}") for i in range(2)]
        for p in range(NPOS // 512):
            pos = slice(512 * p, 512 * (p + 1))
            yps = []
            for eh in range(2):
                ps = psum.tile([P, 512], F32, tag="proj", bufs=3, name=f"cps{eh}")
                ce = slice(128 * eh, 128 * eh + 128)
                nc.tensor.matmul(ps[:], lhsT=convA[:, ce], rhs=acc_f[:, pos],
                                 start=True, stop=False)
                nc.tensor.matmul(ps[:], lhsT=convX[:, ce], rhs=xp_f[:, pos],
                                 start=False, stop=True)
                yps.append(ps)
            # evacuate only after both halves' matmuls have read acc/xp
            for eh in range(2):
                nc.scalar.activation(y_f[eh][:, pos], yps[eh][:], AF.Relu)
                nc.vector.bn_stats(bnb[eh][:, p, :], y_f[eh][:, pos])

        mv = stats.tile([P, 2, 2], F32)
        for eh in range(2):
            nc.vector.bn_aggr(mv[:, eh, :], bnb[eh][:])
        cc_in = stats.tile([P, 4], F32)
        for eh in range(2):
            # [mean, E[y^2]] per half; E[y^2] = var + mean^2
            nc.vector.tensor_copy(cc_in[:, 2 * eh:2 * eh + 1], mv[:, eh, 0:1])
            nc.vector.scalar_tensor_tensor(
                cc_in[:, 2 * eh + 1:2 * eh + 2],
                in0=mv[:, eh, 0:1], scalar=mv[:, eh, 0:1], in1=mv[:, eh, 1:2],
                op0=AluOpType.mult, op1=AluOpType.add)
        nc.sync.dma_start(out=stats_in_d, in_=cc_in[:])
        nc.gpsimd.collective_compute(
            "AllReduce", AluOpType.add,
            replica_groups=[list(range(NCORES))],
            ins=[stats_in_d], outs=[stats_out_d])
        gst = stats.tile([P, 4], F32)
        nc.sync.dma_start(out=gst[:], in_=stats_out_d)

        # ---- phase 5: BN affine coefficients ----
        t0 = stats.tile([P, 4], F32)
        nc.vector.tensor_scalar_mul(t0[:], gst[:], 1.0 / NCORES)
        t0v = t0[:].rearrange("c (e two) -> c e two", two=2)
        m2 = stats.tile([P, 2], F32)
        veps = stats.tile([P, 2], F32)
        for eh in range(2):
            nc.vector.tensor_mul(m2[:, eh:eh + 1], t0v[:, eh, 0:1], t0v[:, eh, 0:1])
            nc.vector.scalar_tensor_tensor(
                veps[:, eh:eh + 1],
                in0=t0v[:, eh, 1:2], scalar=BN_EPS, in1=m2[:, eh:eh + 1],
                op0=AluOpType.add, op1=AluOpType.subtract)
        sd = stats.tile([P, 2], F32)
        nc.scalar.sqrt(sd[:], veps[:])
        rstd = stats.tile([P, 2], F32)
        nc.vector.reciprocal(rstd[:], sd[:])
        scl = stats.tile([P, 2], F32)
        nc.vector.tensor_mul(scl[:], gamma2[:], rstd[:])
        msc = stats.tile([P, 2], F32)
        means = stats.tile([P, 2], F32)
        nc.vector.tensor_copy(means[:, 0:1], t0v[:, 0, 0:1])
        nc.vector.tensor_copy(means[:, 1:2], t0v[:, 1, 0:1])
        nc.vector.tensor_mul(msc[:], means[:], scl[:])
        shift = stats.tile([P, 2], F32)
        nc.vector.tensor_sub(shift[:], beta2[:], msc[:])

        # ---- phase 6: BN affine + output (bf16) ----
        out_r = out_d.rearrange("(two c) h w -> two c (h w)", two=2)
        for p in range(NPOS // 2048):
            pos = slice(2048 * p, 2048 * (p + 1))
            for eh in range(2):
                yo = work.tile([P, 2048], BF16, tag=f"yo{eh}", bufs=2)
                nc.vector.tensor_scalar(
                    yo[:], y_f[eh][:, pos], scl[:, eh:eh + 1], shift[:, eh:eh + 1],
                    op0=AluOpType.mult, op1=AluOpType.add)
                nc.sync.dma_start(out=out_r[eh, :, pos], in_=yo[:])

    nc.finalize()
    return nc


def _get_program():
    if "nc" not in _CACHE:
        _CACHE["nc"] = _build_program()
    return _CACHE["nc"]


def _make_in_maps(x, Wq_h, Wkv_h, Wout_h, bout_h, Wq_w, Wkv_w, Wout_w, bout_w,
                  conv_w, gamma, beta):
    import ml_dtypes
    f = np.float32
    bf = ml_dtypes.bfloat16
    shared = {
        "wq_w": np.ascontiguousarray(np.asarray(Wq_w, f).astype(bf)),
        "wk_w": np.ascontiguousarray(np.asarray(Wkv_w, f)[:, :C].astype(bf)),
        "wv_w": np.ascontiguousarray(np.asarray(Wkv_w, f)[:, C:].astype(bf)),
        "wo_w": np.ascontiguousarray(np.asarray(Wout_w, f).astype(bf)),
        "wq_h": np.ascontiguousarray(np.asarray(Wq_h, f).astype(bf)),
        "wk_h": np.ascontiguousarray(np.asarray(Wkv_h, f)[:, :C].astype(bf)),
        "wv_h": np.ascontiguousarray(np.asarray(Wkv_h, f)[:, C:].astype(bf)),
        "wo_h": np.ascontiguousarray(np.asarray(Wout_h, f).astype(bf)),
        "bsum": np.ascontiguousarray((np.asarray(bout_h, f) + np.asarray(bout_w, f)).reshape(C, 1)),
        "convA": np.ascontiguousarray(np.asarray(conv_w, f)[:C, :].astype(bf)),
        "convX": np.ascontiguousarray(np.asarray(conv_w, f)[C:, :].astype(bf)),
        "gamma2": np.ascontiguousarray(np.asarray(gamma, f).reshape(2, C).T),
        "beta2": np.ascontiguousarray(np.asarray(beta, f).reshape(2, C).T),
    }
    xb = np.asarray(x, f).astype(bf)
    return [{**shared, "x": np.ascontiguousarray(xb[b])} for b in range(B)]


def run(trace=False, **inputs):
    from concourse.bass_utils import run_bass_kernel_spmd

    nc = _get_program()
    in_maps = _make_in_maps(**inputs)
    res = run_bass_kernel_spmd(nc, in_maps, list(range(NCORES)), trace=trace)
    out = np.stack([np.asarray(res.results[b]["out"], np.float32) for b in range(B)],
                   axis=0)
    return out, res


def kernel(**inputs):
    out, _ = run(trace=False, **inputs)
    return out


# revision 27
# speedup vs baseline: 1.1749x; 1.1749x over previous
"""Trainium2 Bass kernel for BlockAxialDown (maxpool + axial attention + 1x1 conv + batchnorm).

Contract: kernel(**inputs) takes FULL unsharded inputs, returns FULL output.
Sharding: data-parallel over batch B=8 across 8 NeuronCores (1 image/core);
BatchNorm batch stats combined with a tiny (128,4) AllReduce; weights replicated.

Transposed-attention dataflow: dots are computed as dotsT[j,i] = k^T q so the
attention weights come out already in the (key, query) layout the AV matmul
needs -- no transposes anywhere.  Softmax denominators are produced by a
ones-matmul whose output is naturally broadcast across all 128 partitions;
normalization is one reciprocal + one multiply fused into the PSUM drain.
Conv runs once with y kept on-chip (bf16), BN affine applied after the
stats AllReduce.  Output returned as bf16, cast to f32 on host.
"""

import sys

import numpy as np

for _p in ("/opt/trn_rl_repo", "/root/.axon_site/_ro/trn_rl_repo"):
    if _p not in sys.path:
        sys.path.append(_p)

B, C, H, W = 8, 128, 256, 256
H2, W2 = 128, 128
E = 2 * C
NPOS = H2 * W2
NCORES = 8
BN_EPS = 1e-5
DH = C // 2
SCALE = DH ** -0.5

_CACHE = {}


def _build_program():
    import concourse.tile as tile
    from concourse import bacc, mybir
    from concourse.alu_op_type import AluOpType
    from contextlib import ExitStack

    F32 = mybir.dt.float32
    BF16 = mybir.dt.bfloat16
    AF = mybir.ActivationFunctionType
    P = 128

    nc = bacc.Bacc("TRN2", target_bir_lowering=False, debug=False, num_devices=NCORES)

    # ---- DRAM I/O ----
    x_d = nc.dram_tensor("x", [C, H, W], BF16, kind="ExternalInput").ap()
    wq_w_d = nc.dram_tensor("wq_w", [C, C], BF16, kind="ExternalInput").ap()
    wk_w_d = nc.dram_tensor("wk_w", [C, C], BF16, kind="ExternalInput").ap()
    wv_w_d = nc.dram_tensor("wv_w", [C, C], BF16, kind="ExternalInput").ap()
    wo_w_d = nc.dram_tensor("wo_w", [C, C], BF16, kind="ExternalInput").ap()
    wq_h_d = nc.dram_tensor("wq_h", [C, C], BF16, kind="ExternalInput").ap()
    wk_h_d = nc.dram_tensor("wk_h", [C, C], BF16, kind="ExternalInput").ap()
    wv_h_d = nc.dram_tensor("wv_h", [C, C], BF16, kind="ExternalInput").ap()
    wo_h_d = nc.dram_tensor("wo_h", [C, C], BF16, kind="ExternalInput").ap()
    bsum_d = nc.dram_tensor("bsum", [C, 1], F32, kind="ExternalInput").ap()
    convA_d = nc.dram_tensor("convA", [C, E], BF16, kind="ExternalInput").ap()
    convX_d = nc.dram_tensor("convX", [C, E], BF16, kind="ExternalInput").ap()
    gamma2_d = nc.dram_tensor("gamma2", [C, 2], F32, kind="ExternalInput").ap()
    beta2_d = nc.dram_tensor("beta2", [C, 2], F32, kind="ExternalInput").ap()
    out_d = nc.dram_tensor("out", [E, H2, W2], BF16, kind="ExternalOutput").ap()
    stats_in_d = nc.dram_tensor("stats_in", [P, 4], F32).ap()
    stats_out_d = nc.dram_tensor("stats_out", [NCORES * P, 4], F32,
                                 addr_space="Shared").ap()

    with tile.TileContext(nc) as tc, ExitStack() as ctx:
        const = ctx.enter_context(tc.tile_pool(name="const", bufs=1))
        cube = ctx.enter_context(tc.tile_pool(name="cube", bufs=1))
        stage = ctx.enter_context(tc.tile_pool(name="stage", bufs=3))
        work = ctx.enter_context(tc.tile_pool(name="work", bufs=3))
        stats = ctx.enter_context(tc.tile_pool(name="stats", bufs=1))
        psum = ctx.enter_context(tc.tile_pool(name="psum", bufs=1, space="PSUM"))

        # ---- constants ----
        def cload(name, ap_d, shape, dt):
            t = const.tile(shape, dt, name=name)
            nc.sync.dma_start(out=t[:], in_=ap_d)
            return t

        wq_w = cload("wq_w_t", wq_w_d, [C, C], BF16)
        wk_w = cload("wk_w_t", wk_w_d, [C, C], BF16)
        wv_w = cload("wv_w_t", wv_w_d, [C, C], BF16)
        wo_w = cload("wo_w_t", wo_w_d, [C, C], BF16)
        wq_h = cload("wq_h_t", wq_h_d, [C, C], BF16)
        wk_h = cload("wk_h_t", wk_h_d, [C, C], BF16)
        wv_h = cload("wv_h_t", wv_h_d, [C, C], BF16)
        wo_h = cload("wo_h_t", wo_h_d, [C, C], BF16)
        bsum = cload("bsum_t", bsum_d, [C, 1], F32)
        convA = cload("convA_t", convA_d, [C, E], BF16)
        convX = cload("convX_t", convX_d, [C, E], BF16)
        gamma2 = cload("gamma2_t", gamma2_d, [C, 2], F32)
        beta2 = cload("beta2_t", beta2_d, [C, 2], F32)
        ones64 = const.tile([P, 64], BF16, name="ones64")
        nc.vector.memset(ones64[:], 1.0)

        xp = cube.tile([P, H2, W2], BF16)   # pooled input, channels on partitions
        acc = cube.tile([P, H2, W2], BF16)  # attention output accumulator
        xp_f = xp[:].rearrange("c h w -> c (h w)")
        acc_f = acc[:].rearrange("c h w -> c (h w)")

        # ---- phase 1: load + 2x2 maxpool ----
        xv = x_d.rearrange("c (n r) w -> c n r w", r=16)
        for i in range(H // 16):
            xin = stage.tile([P, 16, W], BF16, tag="xin")
            nc.sync.dma_start(out=xin[:], in_=xv[:, i])
            t = stage.tile([P, 16, W2], BF16, tag="wmax")
            xin4 = xin[:].rearrange("c r (w two) -> c r w two", two=2)
            nc.vector.tensor_max(t[:], xin4[:, :, :, 0], xin4[:, :, :, 1])
            t4 = t[:].rearrange("c (r2 two) w -> c r2 two w", two=2)
            nc.vector.tensor_max(xp[:, 8 * i:8 * i + 8, :], t4[:, :, 0, :], t4[:, :, 1, :])

        # ---- axial attention, 2-stage software pipeline over 64 groups ----
        # Front half of group g (projections, dots, exp) is emitted together
        # with the back half of group g-1 (denominators, AV, out-proj, acc) so
        # the in-order ACT/DVE queues never block on the current group's tail.
        NG = H2 // 4  # 32 groups per direction

        def grp(g):
            """(rhs_g, weights..., is_w_direction) for global group index g."""
            if g < NG:
                return (xp[:, 4 * g:4 * g + 4, :], wq_w, wk_w, wv_w, wo_w, True, g)
            gg = g - NG
            rhs = xp[:, :, 4 * gg:4 * gg + 4].rearrange("c h w -> c w h")
            return (rhs, wq_h, wk_h, wv_h, wo_h, False, gg)

        def front(g):
            rhs_g, wq, wk, wv, wo, _, _ = grp(g)
            qg_ps = psum.tile([P, 512], F32, tag="proj", bufs=3, name="qg_ps")
            nc.tensor.matmul(qg_ps[:], lhsT=wq[:], rhs=rhs_g, start=True, stop=True)
            kg_ps = psum.tile([P, 512], F32, tag="proj", bufs=3, name="kg_ps")
            nc.tensor.matmul(kg_ps[:], lhsT=wk[:], rhs=rhs_g, start=True, stop=True)
            qg = work.tile([P, 512], BF16, tag="qg")
            nc.scalar.copy(qg[:], qg_ps[:])
            kg = work.tile([P, 512], BF16, tag="kg")
            nc.vector.tensor_copy(kg[:], kg_ps[:])
            v_ps = psum.tile([P, 512], F32, tag="v", name="v_ps")
            for s in range(4):
                nc.tensor.matmul(v_ps[:, 128 * s:128 * s + 128], lhsT=rhs_g[:, s, :],
                                 rhs=wv[:], start=True, stop=True)
            vs = work.tile([P, 512], BF16, tag="vs")
            nc.scalar.copy(vs[:], v_ps[:])
            # dotsT[j, i] per (slice, head), head-major columns: col = 512*h + 128*s
            dT = psum.tile([P, 1024], F32, tag="dots", name="dT")
            for s in range(4):
                cs = slice(128 * s, 128 * s + 128)
                for h in range(2):
                    hp = slice(64 * h, 64 * h + 64)
                    nc.tensor.matmul(dT[:, 512 * h + 128 * s:512 * h + 128 * s + 128],
                                     lhsT=kg[hp, cs], rhs=qg[hp, cs],
                                     start=True, stop=True)
            e = work.tile([P, 1024], BF16, tag="e")
            nc.scalar.activation(e[:], dT[:], AF.Exp, scale=SCALE)
            return e, vs

        def back(g, e, vs):
            rhs_g, wq, wk, wv, wo, is_w, gg = grp(g)
            # softmax denominators, broadcast across partitions by the matmul
            bc = psum.tile([P, 512], F32, tag="bc", name="bc")
            nc.tensor.matmul(bc[0:64, :], lhsT=ones64[:], rhs=e[:, 0:512],
                             start=True, stop=True)
            nc.tensor.matmul(bc[64:128, :], lhsT=ones64[:], rhs=e[:, 512:1024],
                             start=True, stop=True, tile_position=(0, 64))
            rcp = work.tile([P, 512], F32, tag="rcp")
            nc.vector.reciprocal_approx_fast(rcp[:], bc[:])
            # attn @ v (unnormalized), then normalize while draining PSUM
            oT = psum.tile([P, 512], F32, tag="oT", name="oT")
            for s in range(4):
                nc.tensor.matmul(
                    oT[0:64, 128 * s:128 * s + 128],
                    lhsT=vs[:, 128 * s:128 * s + 64],
                    rhs=e[:, 128 * s:128 * s + 128],
                    start=True, stop=True)
                nc.tensor.matmul(
                    oT[64:128, 128 * s:128 * s + 128],
                    lhsT=vs[:, 128 * s + 64:128 * s + 128],
                    rhs=e[:, 512 + 128 * s:512 + 128 * s + 128],
                    start=True, stop=True, tile_position=(0, 64))
            og = work.tile([P, 512], BF16, tag="og")
            nc.vector.tensor_mul(og[:], oT[:], rcp[:])
            yg = psum.tile([P, 512], F32, tag="proj", bufs=3, name="yg_ps")
            nc.tensor.matmul(yg[:], lhsT=wo[:], rhs=og[:], start=True, stop=True)
            if is_w:
                # acc = yg_w + (bout_h + bout_w), contiguous write
                nc.scalar.activation(acc_f[:, 512 * gg:512 * (gg + 1)], yg[:],
                                     AF.Identity, bias=bsum[:, 0:1], scale=1.0)
            else:
                # accumulate transposed: acc[:, h, w] += yg[:, (s=w, i=h)]
                acc_sl = acc[:, :, 4 * gg:4 * gg + 4]
                yg_r = yg[:].rearrange("c (s i) -> c i s", s=4)
                nc.vector.tensor_add(acc_sl, acc_sl, yg_r)

        LAG = 2
        pend = []
        for g in range(2 * NG):
            pend.append((g, front(g)))
            if len(pend) > LAG:
                pg, pe = pend.pop(0)
                back(pg, *pe)
        for pg, pe in pend:
            back(pg, *pe)

        # ---- phase 3.5: relu over acc ----
        for j in range(4):
            sl = acc_f[:, 4096 * j:4096 * (j + 1)]
            nc.vector.tensor_scalar_max(sl, sl, 0.0)

        # ---- phase 4: conv (once), relu evac to y (over acc/xp), stats ----
        # y half 0 overwrites acc chunk-by-chunk, half 1 overwrites xp.
        y_f = [acc_f, xp_f]
        bnb = [stats.tile([P, 32, 6], F32, name=f"bnb{i}") for i in range(2)]
        for p in range(NPOS // 512):
            pos = slice(512 * p, 512 * (p + 1))
            yps = []
            for eh in range(2):
                ps = psum.tile([P, 512], F32, tag="proj", bufs=3, name=f"cps{eh}")
                ce = slice(128 * eh, 128 * eh + 128)
                nc.tensor.matmul(ps[:], lhsT=convA[:, ce], rhs=acc_f[:, pos],
                                 start=True, stop=False)
                nc.tensor.matmul(ps[:], lhsT=convX[:, ce], rhs=xp_f[:, pos],
                                 start=False, stop=True)
                yps.append(ps)
            # evacuate only after both halves' matmuls have read acc/xp
            for eh in range(2):
                nc.scalar.activation(y_f[eh][:, pos], yps[eh][:], AF.Relu)
                nc.vector.bn_stats(bnb[eh][:, p, :], y_f[eh][:, pos])

        mv = stats.tile([P, 2, 2], F32)
        for eh in range(2):
            nc.vector.bn_aggr(mv[:, eh, :], bnb[eh][:])
        cc_in = stats.tile([P, 4], F32)
        for eh in range(2):
            # [mean, E[y^2]] per half; E[y^2] = var + mean^2
            nc.vector.tensor_copy(cc_in[:, 2 * eh:2 * eh + 1], mv[:, eh, 0:1])
            nc.vector.scalar_tensor_tensor(
                cc_in[:, 2 * eh + 1:2 * eh + 2],
                in0=mv[:, eh, 0:1], scalar=mv[:, eh, 0:1], in1=mv[:, eh, 1:2],
                op0=AluOpType.mult, op1=AluOpType.add)
        nc.sync.dma_start(out=stats_in_d, in_=cc_in[:])
        # AllGather (no CCE compute in the ring) + local tree-sum; gathered
        # blocks concatenate along the partition axis: block r = rows r*128..
        nc.gpsimd.collective_compute(
            "AllGather", AluOpType.bypass,
            replica_groups=[list(range(NCORES))],
            ins=[stats_in_d], outs=[stats_out_d])
        gall = stats.tile([P, NCORES, 4], F32)
        nc.sync.dma_start(out=gall[:],
                          in_=stats_out_d.rearrange("(r c) f -> c r f", r=NCORES))
        gv = gall[:]
        for step in (4, 2, 1):
            for r in range(step):
                nc.vector.tensor_add(gv[:, r], gv[:, r], gv[:, r + step])
        gst = stats.tile([P, 4], F32)
        nc.vector.tensor_copy(gst[:], gv[:, 0])

        # ---- phase 5: BN affine coefficients ----
        t0 = stats.tile([P, 4], F32)
        nc.vector.tensor_scalar_mul(t0[:], gst[:], 1.0 / NCORES)
        t0v = t0[:].rearrange("c (e two) -> c e two", two=2)
        m2 = stats.tile([P, 2], F32)
        veps = stats.tile([P, 2], F32)
        for eh in range(2):
            nc.vector.tensor_mul(m2[:, eh:eh + 1], t0v[:, eh, 0:1], t0v[:, eh, 0:1])
            nc.vector.scalar_tensor_tensor(
                veps[:, eh:eh + 1],
                in0=t0v[:, eh, 1:2], scalar=BN_EPS, in1=m2[:, eh:eh + 1],
                op0=AluOpType.add, op1=AluOpType.subtract)
        sd = stats.tile([P, 2], F32)
        nc.scalar.sqrt(sd[:], veps[:])
        rstd = stats.tile([P, 2], F32)
        nc.vector.reciprocal(rstd[:], sd[:])
        scl = stats.tile([P, 2], F32)
        nc.vector.tensor_mul(scl[:], gamma2[:], rstd[:])
        msc = stats.tile([P, 2], F32)
        means = stats.tile([P, 2], F32)
        nc.vector.tensor_copy(means[:, 0:1], t0v[:, 0, 0:1])
        nc.vector.tensor_copy(means[:, 1:2], t0v[:, 1, 0:1])
        nc.vector.tensor_mul(msc[:], means[:], scl[:])
        shift = stats.tile([P, 2], F32)
        nc.vector.tensor_sub(shift[:], beta2[:], msc[:])

        # ---- phase 6: BN affine + output (bf16) ----
        out_r = out_d.rearrange("(two c) h w -> two c (h w)", two=2)
        for p in range(NPOS // 2048):
            pos = slice(2048 * p, 2048 * (p + 1))
            for eh in range(2):
                yo = work.tile([P, 2048], BF16, tag=f"yo{eh}", bufs=2)
                nc.vector.tensor_scalar(
                    yo[:], y_f[eh][:, pos], scl[:, eh:eh + 1], shift[:, eh:eh + 1],
                    op0=AluOpType.mult, op1=AluOpType.add)
                nc.sync.dma_start(out=out_r[eh, :, pos], in_=yo[:])

    nc.finalize()
    return nc


def _get_program():
    if "nc" not in _CACHE:
        _CACHE["nc"] = _build_program()
    return _CACHE["nc"]


def _make_in_maps(x, Wq_h, Wkv_h, Wout_h, bout_h, Wq_w, Wkv_w, Wout_w, bout_w,
                  conv_w, gamma, beta):
    import ml_dtypes
    f = np.float32
    bf = ml_dtypes.bfloat16
    shared = {
        "wq_w": np.ascontiguousarray(np.asarray(Wq_w, f).astype(bf)),
        "wk_w": np.ascontiguousarray(np.asarray(Wkv_w, f)[:, :C].astype(bf)),
        "wv_w": np.ascontiguousarray(np.asarray(Wkv_w, f)[:, C:].astype(bf)),
        "wo_w": np.ascontiguousarray(np.asarray(Wout_w, f).astype(bf)),
        "wq_h": np.ascontiguousarray(np.asarray(Wq_h, f).astype(bf)),
        "wk_h": np.ascontiguousarray(np.asarray(Wkv_h, f)[:, :C].astype(bf)),
        "wv_h": np.ascontiguousarray(np.asarray(Wkv_h, f)[:, C:].astype(bf)),
        "wo_h": np.ascontiguousarray(np.asarray(Wout_h, f).astype(bf)),
        "bsum": np.ascontiguousarray((np.asarray(bout_h, f) + np.asarray(bout_w, f)).reshape(C, 1)),
        "convA": np.ascontiguousarray(np.asarray(conv_w, f)[:C, :].astype(bf)),
        "convX": np.ascontiguousarray(np.asarray(conv_w, f)[C:, :].astype(bf)),
        "gamma2": np.ascontiguousarray(np.asarray(gamma, f).reshape(2, C).T),
        "beta2": np.ascontiguousarray(np.asarray(beta, f).reshape(2, C).T),
    }
    xb = np.asarray(x, f).astype(bf)
    return [{**shared, "x": np.ascontiguousarray(xb[b])} for b in range(B)]


def run(trace=False, **inputs):
    from concourse.bass_utils import run_bass_kernel_spmd

    nc = _get_program()
    in_maps = _make_in_maps(**inputs)
    res = run_bass_kernel_spmd(nc, in_maps, list(range(NCORES)), trace=trace)
    out = np.stack([np.asarray(res.results[b]["out"], np.float32) for b in range(B)],
                   axis=0)
    return out, res


def kernel(**inputs):
    out, _ = run(trace=False, **inputs)
    return out
